# revision 4
# baseline (speedup 1.0000x reference)
"""AI4DEM contact-force kernel for 8 TRN2 NeuronCores.

Physics (from the reference): two particle layers on a 128^3 grid; for each
layer n, accumulate spring-damper contact forces from both layers over a
5x5x5 neighborhood of rolls, then integrate velocities.  Because
cell_size == particle_size == 0.1 and particle jitter < cell_size, any
offset with |shift| >= 2 in some axis can never produce a contact
(dist > PS provably), so the 125-point stencil reduces exactly to 3x3x3.
Roll wrap-around contributions are likewise provably zero (positions
differ by ~12.7), so the stencil is a pure local halo-1 stencil with
far-value sentinels at the global boundary.

Distribution: shard z (first spatial axis) across the 8 cores, 16 planes
each.  Layer-1 forces depend on layer-0's *updated* velocities, so each
core takes a halo of 2 input planes per side (inputs host-padded with
sentinel planes so all cores run an identical program) and no inter-core
communication is needed at all.

Layout on core: partition dim = y (128 rows), free dims = (z-chunk, x).
y-shifted stencil reads are materialized as 3 y-offset DMA loads from the
host-padded (y=130) arrays; z and x shifts are free-dim window offsets.
1/dist and 1/dist^2 are computed as Exp(-0.5*Ln(s)) / Exp(-Ln(s)+ln(ETA))
on the Scalar engine (one activation-table set; DVE reciprocal is ~6x
slower than tensor ops).
"""

import math
import sys

import numpy as np

sys.path.insert(0, "/opt/trn_rl_repo")

D = 128
CELL = 0.1
PS = 0.1
KN = 6.0e6
_ALPHA = -math.log(0.5) / math.pi
_GAMMA = _ALPHA / math.sqrt(_ALPHA**2 + 1.0)
PM = 4.0 / 3.0 * 3.1415 * CELL**3 * 2700.0
ETA = 2.0 * _GAMMA * math.sqrt(KN * PM)
DT = 1e-4
SENT = 1.0e3      # far-value sentinel for positions at global boundaries
NCORES = 8
ZP = D // NCORES  # 16 output planes per core
NZIN = ZP + 4     # input planes per core (halo 2 each side)
NY = D + 2        # host-padded y extent
NX = D + 2        # host-padded x extent

# boundary-force constants
BLO_HI = 1.5 * PS                         # lo band upper edge
BHI_TH = D * CELL - 0.5 * PS - CELL       # hi threshold (12.65)
GRAV = -9.8 * PM

ZC_MAX = 6
ZCP2 = ZC_MAX + 2

# 27 shifts (sz, sy, sx); contributions with any |shift|>=2 are provably zero
SHIFTS = [(a, b, c) for a in (-1, 0, 1) for b in (-1, 0, 1) for c in (-1, 0, 1)]

FIELD_NAMES = ["xg", "yg", "zg", "vx", "vy", "vz"]

_compiled = None


def _build():
    from contextlib import ExitStack
    from concourse import bacc, tile, mybir

    dt = mybir.dt.float32
    A = mybir.AluOpType
    AF = mybir.ActivationFunctionType

    nc = bacc.Bacc("TRN2", target_bir_lowering=False, debug=False)

    # --- DRAM I/O ------------------------------------------------------
    ext = {}
    for f in FIELD_NAMES + ["mk"]:
        ext[f] = nc.dram_tensor(f, [2, NZIN, NY, NX], dt, kind="ExternalInput").ap()
    out_ext = nc.dram_tensor("out", [6, ZP, D, D], dt, kind="ExternalOutput").ap()
    # layer-0 updated velocities, round-tripped through DRAM for phase 1.
    # 18 planes: local z in [1, 19).
    scr = nc.dram_tensor("v0s", [3, ZP + 2, NY, NX], dt).ap()

    with tile.TileContext(nc) as tc:
        with ExitStack() as ctx:
            pool = ctx.enter_context(tc.tile_pool(name="sbuf", bufs=1))

            # --- static tiles -----------------------------------------
            # m-side stencil source fields, 3 y-variants each
            mt = {(f, v): pool.tile([D, ZCP2, NX], dt, name=f"mt_{f}_{v}", tag=f"mt_{f}_{v}")
                  for f in range(6) for v in (-1, 0, 1)}
            # n-side aligned fields (positions + velocities of layer n)
            nt = [pool.tile([D, ZCP2, NX], dt, name=f"nt_{f}", tag=f"nt_{f}") for f in range(6)]
            mk = pool.tile([D, ZC_MAX, NX], dt, name="mk", tag="mk")

            inter = {}
            for tg in ["dX", "dY", "dZ", "QA", "QB", "QC", "S", "S2", "LN",
                       "R", "R2", "Aa", "G", "VA", "VB", "VC", "P", "T", "C"]:
                inter[tg] = pool.tile([D, ZC_MAX, D], dt, name=tg, tag=tg)
            FX = pool.tile([D, ZC_MAX, D], dt, name="FX", tag="FX")
            FY = pool.tile([D, ZC_MAX, D], dt, name="FY", tag="FY")
            FZ = pool.tile([D, ZC_MAX, D], dt, name="FZ", tag="FZ")
            VN = [pool.tile([D, ZC_MAX, NX], dt, name=f"VN{i}", tag=f"VN{i}") for i in range(3)]

            b_eps = pool.tile([D, 1], dt, name="b_eps", tag="b_eps")
            b_lneta = pool.tile([D, 1], dt, name="b_lneta", tag="b_lneta")
            zt = pool.tile([D, NX], dt, name="zt", tag="zt")
            nc.vector.memset(b_eps[:], 1e-8)
            nc.vector.memset(b_lneta[:], math.log(ETA))
            nc.vector.memset(zt[:], 0.0)

            # zero the y-pad rows of the scratch (x pads come from VN pads)
            for f in range(3):
                nc.sync.dma_start(scr[f, :, 0, :], zt[0:ZP + 2, :])
                nc.sync.dma_start(scr[f, :, NY - 1, :], zt[0:ZP + 2, :])

            def load(tile_t, src_ap):
                nc.sync.dma_start(tile_t, src_ap.rearrange("z y x -> y z x"))

            def emit_combo(zc, sh, nsrc, msrc):
                """One (shift, m) term: accumulate force on the out-window."""
                shz, shy, shx = sh
                v = -shy
                z0, x0 = 1 - shz, 1 - shx
                # m-side windowed reads
                ms = [msrc[f](v)[:, z0:z0 + zc, x0:x0 + D] for f in range(6)]
                # n-side aligned reads
                ns = [nsrc[f][:, 1:1 + zc, 1:1 + D] for f in range(6)]
                w = slice(0, zc)
                dX = inter["dX"][:, w, :]
                dY = inter["dY"][:, w, :]
                dZ = inter["dZ"][:, w, :]
                QA = inter["QA"][:, w, :]
                QB = inter["QB"][:, w, :]
                QC = inter["QC"][:, w, :]
                S = inter["S"][:, w, :]
                S2 = inter["S2"][:, w, :]
                LN = inter["LN"][:, w, :]
                R = inter["R"][:, w, :]
                R2 = inter["R2"][:, w, :]
                Aa = inter["Aa"][:, w, :]
                G = inter["G"][:, w, :]
                VA = inter["VA"][:, w, :]
                VB = inter["VB"][:, w, :]
                VC = inter["VC"][:, w, :]
                P = inter["P"][:, w, :]
                T = inter["T"][:, w, :]
                C = inter["C"][:, w, :]
                fx = FX[:, w, :]
                fy = FY[:, w, :]
                fz = FZ[:, w, :]

                tt = nc.vector.tensor_tensor
                ts = nc.vector.tensor_scalar
                act = nc.scalar.activation

                tt(dX, ns[0], ms[0], A.subtract)
                tt(dY, ns[1], ms[1], A.subtract)
                tt(dZ, ns[2], ms[2], A.subtract)
                act(QA, dX, AF.Square)
                act(QB, dY, AF.Square)
                act(QC, dZ, AF.Square)
                tt(S, QA, QB, A.add)
                tt(S2, S, QC, A.add)
                act(LN, S2, AF.Ln, bias=b_eps[:], scale=1.0)
                act(R, LN, AF.Exp, bias=0.0, scale=-0.5)
                act(R2, LN, AF.Exp, bias=b_lneta[:], scale=-1.0)
                ts(Aa, R, -KN * PS, KN, A.mult, A.add)
                # gate on s directly: [dist < PS] == [s < PS^2] to ~1 ulp,
                # avoiding the ~1e-5 relative error of the ln/exp path
                ts(G, S2, PS * PS, None, A.is_lt)
                tt(VA, ns[3], ms[3], A.subtract)
                tt(VB, ns[4], ms[4], A.subtract)
                tt(VC, ns[5], ms[5], A.subtract)
                tt(P, VA, dX, A.mult)
                tt(T, VB, dY, A.mult)
                tt(P, P, T, A.add)
                tt(T, VC, dZ, A.mult)
                tt(P, P, T, A.add)
                tt(T, P, R2, A.mult)       # damping term * r^2
                tt(C, Aa, T, A.add)
                tt(C, C, G, A.mult)        # gate
                tt(T, C, dX, A.mult)
                tt(fx, fx, T, A.add)
                tt(T, C, dY, A.mult)
                tt(fy, fy, T, A.add)
                tt(T, C, dZ, A.mult)
                tt(fz, fz, T, A.add)

            def boundary_and_update(zc, nsrc, gravity):
                """fb per component, then VN = v + DT/PM*mask*(-f+fb[+grav])."""
                tt = nc.vector.tensor_tensor
                ts = nc.vector.tensor_scalar
                w = slice(0, zc)
                mkw = mk[:, w, 1:1 + D]
                Aa = inter["Aa"][:, w, :]
                G = inter["G"][:, w, :]
                P = inter["P"][:, w, :]
                T = inter["T"][:, w, :]
                C = inter["C"][:, w, :]
                S = inter["S"][:, w, :]
                for comp, (FF, grav) in enumerate(
                        [(FX, 0.0), (FY, 0.0), (FZ, gravity)]):
                    p = nsrc[comp][:, 1:1 + zc, 1:1 + D]
                    vv = nsrc[3 + comp][:, 1:1 + zc, 1:1 + D]
                    f = FF[:, w, :]
                    # lo = (p > PS) & (p < 1.5 PS); hi = (p > 12.65)
                    ts(Aa, p, PS, None, A.is_gt)
                    ts(G, p, BLO_HI, None, A.is_lt)
                    tt(Aa, Aa, G, A.mult)            # lo
                    ts(G, p, BHI_TH, None, A.is_gt)  # hi
                    # fb = KN*lo*(1.5PS - p) - KN*hi*(p - 12.65) - ETA*v*(lo+hi)
                    ts(T, p, -KN, KN * BLO_HI, A.mult, A.add)
                    tt(T, T, Aa, A.mult)             # spring lo
                    ts(C, p, -KN, KN * BHI_TH, A.mult, A.add)
                    tt(C, C, G, A.mult)              # spring hi (already -KN*(p-th))
                    tt(T, T, C, A.add)
                    tt(Aa, Aa, G, A.add)             # lo + hi
                    tt(P, vv, Aa, A.mult)
                    ts(P, P, -ETA, None, A.mult)
                    tt(T, T, P, A.add)               # fb complete
                    # update
                    tt(S, T, f, A.subtract)          # fb - f
                    if grav != 0.0:
                        ts(S, S, 1.0, grav, A.mult, A.add)
                    tt(S, S, mkw, A.mult)
                    ts(S, S, DT / PM, None, A.mult)
                    vn = VN[comp][:, w, 1:1 + D]
                    tt(vn, vv, S, A.add)

            # ================= PHASE 0 (layer n = 0) ===================
            p0_chunks = [(1, 6), (7, 6), (13, 6)]  # v0new local planes [1,19)
            for (w0, zc) in p0_chunks:
                zlo, zhi = w0 - 1, w0 + zc + 1
                # n-side: layer 0 aligned
                for f in range(6):
                    load(nt[f][:, 0:zc + 2, :], ext[FIELD_NAMES[f]][0, zlo:zhi, 1:1 + D, :])
                load(mk[:, 0:zc, :], ext["mk"][0, w0:w0 + zc, 1:1 + D, :])
                nc.gpsimd.memset(FX[:, 0:zc, :], 0.0)
                nc.gpsimd.memset(FY[:, 0:zc, :], 0.0)
                nc.gpsimd.memset(FZ[:, 0:zc, :], 0.0)

                nsrc = [nt[f] for f in range(6)]
                # --- m = 0 (same layer): var0 tiles are the nt tiles
                for f in range(6):
                    for v in (-1, 1):
                        load(mt[(f, v)][:, 0:zc + 2, :],
                             ext[FIELD_NAMES[f]][0, zlo:zhi, 1 + v:1 + v + D, :])
                msrc0 = [
                    (lambda f_: (lambda v: nt[f_] if v == 0 else mt[(f_, v)]))(f)
                    for f in range(6)
                ]
                for sh in SHIFTS:
                    if sh == (0, 0, 0):
                        continue
                    emit_combo(zc, sh, nsrc, msrc0)
                # --- m = 1 (other layer)
                for f in range(6):
                    for v in (-1, 0, 1):
                        load(mt[(f, v)][:, 0:zc + 2, :],
                             ext[FIELD_NAMES[f]][1, zlo:zhi, 1 + v:1 + v + D, :])
                msrc1 = [(lambda f_: (lambda v: mt[(f_, v)]))(f) for f in range(6)]
                for sh in SHIFTS:
                    emit_combo(zc, sh, nsrc, msrc1)

                for i in range(3):
                    nc.gpsimd.memset(VN[i][:, 0:zc, :], 0.0)
                boundary_and_update(zc, nsrc, GRAV)
                # write v0new chunk to scratch (planes j = local-1)
                for i in range(3):
                    nc.sync.dma_start(
                        scr[i, w0 - 1:w0 - 1 + zc, 1:1 + D, :]
                        .rearrange("z y x -> y z x"),
                        VN[i][:, 0:zc, :])
                # write the output sub-range (local planes [2,18))
                olo = max(w0, 2)
                ohi = min(w0 + zc, 2 + ZP)
                if ohi > olo:
                    for i in range(3):
                        nc.sync.dma_start(
                            out_ext[i, olo - 2:ohi - 2, :, :]
                            .rearrange("z y x -> y z x"),
                            VN[i][:, olo - w0:ohi - w0, 1:1 + D])

            # ================= PHASE 1 (layer n = 1) ===================
            p1_chunks = [(2, 6), (8, 6), (14, 4)]  # out planes local [2,18)
            for (w0, zc) in p1_chunks:
                zlo, zhi = w0 - 1, w0 + zc + 1
                for f in range(6):
                    load(nt[f][:, 0:zc + 2, :], ext[FIELD_NAMES[f]][1, zlo:zhi, 1:1 + D, :])
                load(mk[:, 0:zc, :], ext["mk"][1, w0:w0 + zc, 1:1 + D, :])
                nc.gpsimd.memset(FX[:, 0:zc, :], 0.0)
                nc.gpsimd.memset(FY[:, 0:zc, :], 0.0)
                nc.gpsimd.memset(FZ[:, 0:zc, :], 0.0)

                nsrc = [nt[f] for f in range(6)]
                # --- m = 1 (same layer)
                for f in range(6):
                    for v in (-1, 1):
                        load(mt[(f, v)][:, 0:zc + 2, :],
                             ext[FIELD_NAMES[f]][1, zlo:zhi, 1 + v:1 + v + D, :])
                msrc1 = [
                    (lambda f_: (lambda v: nt[f_] if v == 0 else mt[(f_, v)]))(f)
                    for f in range(6)
                ]
                for sh in SHIFTS:
                    if sh == (0, 0, 0):
                        continue
                    emit_combo(zc, sh, nsrc, msrc1)
                # --- m = 0: positions from inputs, velocities from scratch
                for f in range(3):
                    for v in (-1, 0, 1):
                        load(mt[(f, v)][:, 0:zc + 2, :],
                             ext[FIELD_NAMES[f]][0, zlo:zhi, 1 + v:1 + v + D, :])
                for f in range(3):
                    for v in (-1, 0, 1):
                        load(mt[(3 + f, v)][:, 0:zc + 2, :],
                             scr[f, w0 - 2:w0 + zc, 1 + v:1 + v + D, :])
                msrc0 = [(lambda f_: (lambda v: mt[(f_, v)]))(f) for f in range(6)]
                for sh in SHIFTS:
                    emit_combo(zc, sh, nsrc, msrc0)

                for i in range(3):
                    nc.gpsimd.memset(VN[i][:, 0:zc, :], 0.0)
                boundary_and_update(zc, nsrc, GRAV)
                for i in range(3):
                    nc.sync.dma_start(
                        out_ext[3 + i, w0 - 2:w0 - 2 + zc, :, :]
                        .rearrange("z y x -> y z x"),
                        VN[i][:, 0:zc, 1:1 + D])

    nc.compile()
    return nc


def _get_compiled():
    global _compiled
    if _compiled is None:
        _compiled = _build()
    return _compiled


def _pad_field(a, val):
    # (2,1,1,D,D,D) -> (2, D+4, NY, NX)
    a = np.ascontiguousarray(a.reshape(2, D, D, D), dtype=np.float32)
    return np.pad(a, ((0, 0), (2, 2), (1, 1), (1, 1)), constant_values=val)


def kernel(x_grid, y_grid, z_grid, vx_grid, vy_grid, vz_grid, mask):
    from concourse.bass_utils import run_bass_kernel_spmd

    nc = _get_compiled()

    padded = {
        "xg": _pad_field(x_grid, SENT),
        "yg": _pad_field(y_grid, SENT),
        "zg": _pad_field(z_grid, SENT),
        "vx": _pad_field(vx_grid, 0.0),
        "vy": _pad_field(vy_grid, 0.0),
        "vz": _pad_field(vz_grid, 0.0),
        "mk": _pad_field(mask, 0.0),
    }
    in_maps = []
    for c in range(NCORES):
        z0 = ZP * c
        in_maps.append({k: np.ascontiguousarray(v[:, z0:z0 + NZIN])
                        for k, v in padded.items()})

    res = run_bass_kernel_spmd(nc, in_maps, core_ids=list(range(NCORES)))

    out = np.empty((3, 2, 1, 1, D, D, D), np.float32)
    for c in range(NCORES):
        o = res.results[c]["out"]  # (6, ZP, D, D)
        z0 = ZP * c
        for comp in range(3):
            out[comp, 0, 0, 0, z0:z0 + ZP] = o[comp]
            out[comp, 1, 0, 0, z0:z0 + ZP] = o[3 + comp]
    return out


# revision 7
# speedup vs baseline: 1.0037x; 1.0037x over previous
"""AI4DEM contact-force kernel for 8 TRN2 NeuronCores.

Physics (from the reference): two particle layers on a 128^3 grid; for each
layer n, accumulate spring-damper contact forces from both layers over a
5x5x5 neighborhood of rolls, then integrate velocities.  Because
cell_size == particle_size == 0.1 and particle jitter < cell_size, any
offset with |shift| >= 2 in some axis can never produce a contact
(dist > PS provably), so the 125-point stencil reduces exactly to 3x3x3.
Roll wrap-around contributions are likewise provably zero (positions
differ by ~12.7), so the stencil is a pure local halo-1 stencil with
far-value sentinels at the global boundary.

Distribution: shard z (first spatial axis) across the 8 cores, 16 planes
each.  Layer-1 forces depend on layer-0's *updated* velocities, so each
core takes a halo of 2 input planes per side (inputs host-padded with
sentinel planes so all cores run an identical program) and no inter-core
communication is needed at all.

Layout on core: partition dim = y (128 rows), free dims = (z-chunk, x).
y-shifted stencil reads are materialized as 3 y-offset DMA loads from the
host-padded (y=130) arrays; z and x shifts are free-dim window offsets.
1/dist and 1/dist^2 are computed as Exp(-0.5*Ln(s)) / Exp(-Ln(s)+ln(ETA))
on the Scalar engine (one activation-table set; DVE reciprocal is ~6x
slower than tensor ops).
"""

import math
import sys

import numpy as np

sys.path.insert(0, "/opt/trn_rl_repo")

D = 128
CELL = 0.1
PS = 0.1
KN = 6.0e6
_ALPHA = -math.log(0.5) / math.pi
_GAMMA = _ALPHA / math.sqrt(_ALPHA**2 + 1.0)
PM = 4.0 / 3.0 * 3.1415 * CELL**3 * 2700.0
ETA = 2.0 * _GAMMA * math.sqrt(KN * PM)
DT = 1e-4
SENT = 1.0e3      # far-value sentinel for positions at global boundaries
NCORES = 8
ZP = D // NCORES  # 16 output planes per core
NZIN = ZP + 4     # input planes per core (halo 2 each side)
NY = D + 2        # host-padded y extent
NX = D + 2        # host-padded x extent

# boundary-force constants
BLO_HI = 1.5 * PS                         # lo band upper edge
BHI_TH = D * CELL - 0.5 * PS - CELL       # hi threshold (12.65)
GRAV = -9.8 * PM

ZC_MAX = 6
ZCP2 = ZC_MAX + 2

# 27 shifts (sz, sy, sx); contributions with any |shift|>=2 are provably zero
SHIFTS = [(a, b, c) for a in (-1, 0, 1) for b in (-1, 0, 1) for c in (-1, 0, 1)]

FIELD_NAMES = ["xg", "yg", "zg", "vx", "vy", "vz"]

_compiled = None


def _build():
    from contextlib import ExitStack
    from concourse import bacc, tile, mybir

    dt = mybir.dt.float32
    A = mybir.AluOpType
    AF = mybir.ActivationFunctionType

    nc = bacc.Bacc("TRN2", target_bir_lowering=False, debug=False)

    # --- DRAM I/O ------------------------------------------------------
    ext = {}
    for f in FIELD_NAMES + ["mk"]:
        ext[f] = nc.dram_tensor(f, [2, NZIN, NY, NX], dt, kind="ExternalInput").ap()
    out_ext = nc.dram_tensor("out", [6, ZP, D, D], dt, kind="ExternalOutput").ap()
    # layer-0 updated velocities, round-tripped through DRAM for phase 1.
    # 18 planes: local z in [1, 19).
    scr = nc.dram_tensor("v0s", [3, ZP + 2, NY, NX], dt).ap()

    with tile.TileContext(nc) as tc:
        with ExitStack() as ctx:
            pool = ctx.enter_context(tc.tile_pool(name="sbuf", bufs=1))

            # --- static tiles -----------------------------------------
            # m-side stencil source fields, 3 y-variants each
            mt = {(f, v): pool.tile([D, ZCP2, NX], dt, name=f"mt_{f}_{v}", tag=f"mt_{f}_{v}")
                  for f in range(6) for v in (-1, 0, 1)}
            # n-side aligned fields (positions + velocities of layer n)
            nt = [pool.tile([D, ZCP2, NX], dt, name=f"nt_{f}", tag=f"nt_{f}") for f in range(6)]
            mk = pool.tile([D, ZC_MAX, NX], dt, name="mk", tag="mk")

            inter = {}
            for tg in ["dX", "dY", "dZ", "QA", "QB", "QC", "S", "LN",
                       "R", "R2", "Aa", "G", "VA", "VB", "VC",
                       "P1", "P2", "P3", "C", "TX", "TY", "TZ"]:
                inter[tg] = pool.tile([D, ZC_MAX, D], dt, name=tg, tag=tg)
            FX = pool.tile([D, ZC_MAX, D], dt, name="FX", tag="FX")
            FY = pool.tile([D, ZC_MAX, D], dt, name="FY", tag="FY")
            FZ = pool.tile([D, ZC_MAX, D], dt, name="FZ", tag="FZ")
            VN = [pool.tile([D, ZC_MAX, NX], dt, name=f"VN{i}", tag=f"VN{i}") for i in range(3)]

            b_eps = pool.tile([D, 1], dt, name="b_eps", tag="b_eps")
            b_lneta = pool.tile([D, 1], dt, name="b_lneta", tag="b_lneta")
            zt = pool.tile([D, NX], dt, name="zt", tag="zt")
            nc.vector.memset(b_eps[:], 1e-8)
            nc.vector.memset(b_lneta[:], math.log(ETA))
            nc.vector.memset(zt[:], 0.0)

            # zero the y-pad rows of the scratch (x pads come from VN pads)
            for f in range(3):
                nc.sync.dma_start(scr[f, :, 0, :], zt[0:ZP + 2, :])
                nc.sync.dma_start(scr[f, :, NY - 1, :], zt[0:ZP + 2, :])

            def load(tile_t, src_ap):
                nc.sync.dma_start(tile_t, src_ap.rearrange("z y x -> y z x"))

            def emit_combo(zc, sh, nsrc, msrc):
                """One (shift, m) term: accumulate force on the out-window."""
                shz, shy, shx = sh
                v = -shy
                z0, x0 = 1 - shz, 1 - shx
                # m-side windowed reads
                ms = [msrc[f](v)[:, z0:z0 + zc, x0:x0 + D] for f in range(6)]
                # n-side aligned reads
                ns = [nsrc[f][:, 1:1 + zc, 1:1 + D] for f in range(6)]
                w = slice(0, zc)
                dX = inter["dX"][:, w, :]
                dY = inter["dY"][:, w, :]
                dZ = inter["dZ"][:, w, :]
                QA = inter["QA"][:, w, :]
                QB = inter["QB"][:, w, :]
                QC = inter["QC"][:, w, :]
                S = inter["S"][:, w, :]
                LN = inter["LN"][:, w, :]
                R = inter["R"][:, w, :]
                R2 = inter["R2"][:, w, :]
                Aa = inter["Aa"][:, w, :]
                G = inter["G"][:, w, :]
                VA = inter["VA"][:, w, :]
                VB = inter["VB"][:, w, :]
                VC = inter["VC"][:, w, :]
                P1 = inter["P1"][:, w, :]
                P2 = inter["P2"][:, w, :]
                P3 = inter["P3"][:, w, :]
                C = inter["C"][:, w, :]
                TX = inter["TX"][:, w, :]
                TY = inter["TY"][:, w, :]
                TZ = inter["TZ"][:, w, :]
                fx = FX[:, w, :]
                fy = FY[:, w, :]
                fz = FZ[:, w, :]

                tt = nc.vector.tensor_tensor
                ts = nc.vector.tensor_scalar
                gp = nc.gpsimd.tensor_tensor
                act = nc.scalar.activation

                # deltas: DVE
                tt(dX, ns[0], ms[0], A.subtract)
                tt(dY, ns[1], ms[1], A.subtract)
                tt(dZ, ns[2], ms[2], A.subtract)
                # squares: ACT; sums: GpSimd
                act(QA, dX, AF.Square)
                act(QB, dY, AF.Square)
                act(QC, dZ, AF.Square)
                gp(S, QA, QB, A.add)
                gp(S, S, QC, A.add)
                act(LN, S, AF.Ln, bias=b_eps[:], scale=1.0)
                act(R, LN, AF.Exp, bias=0.0, scale=-0.5)
                act(R2, LN, AF.Exp, bias=b_lneta[:], scale=-1.0)
                ts(Aa, R, -KN * PS, KN, A.mult, A.add)
                # gate on s directly: [dist < PS] == [s < PS^2] to ~1 ulp,
                # avoiding the ~1e-5 relative error of the ln/exp path
                ts(G, S, PS * PS, None, A.is_lt)
                # velocity deltas: GpSimd
                gp(VA, ns[3], ms[3], A.subtract)
                gp(VB, ns[4], ms[4], A.subtract)
                gp(VC, ns[5], ms[5], A.subtract)
                # dot products: DVE; sums: GpSimd
                tt(P1, VA, dX, A.mult)
                tt(P2, VB, dY, A.mult)
                tt(P3, VC, dZ, A.mult)
                gp(P1, P1, P2, A.add)
                gp(P1, P1, P3, A.add)
                tt(P2, P1, R2, A.mult)     # damping term * r^2
                tt(P3, Aa, P2, A.add)
                tt(C, P3, G, A.mult)       # gate
                tt(TX, C, dX, A.mult)
                tt(TY, C, dY, A.mult)
                tt(TZ, C, dZ, A.mult)
                gp(fx, fx, TX, A.add)
                gp(fy, fy, TY, A.add)
                gp(fz, fz, TZ, A.add)

            def boundary_and_update(zc, nsrc, gravity):
                """fb per component, then VN = v + DT/PM*mask*(-f+fb[+grav])."""
                tt = nc.vector.tensor_tensor
                ts = nc.vector.tensor_scalar
                w = slice(0, zc)
                mkw = mk[:, w, 1:1 + D]
                Aa = inter["Aa"][:, w, :]
                G = inter["G"][:, w, :]
                P = inter["P1"][:, w, :]
                T = inter["TX"][:, w, :]
                C = inter["C"][:, w, :]
                S = inter["S"][:, w, :]
                for comp, (FF, grav) in enumerate(
                        [(FX, 0.0), (FY, 0.0), (FZ, gravity)]):
                    p = nsrc[comp][:, 1:1 + zc, 1:1 + D]
                    vv = nsrc[3 + comp][:, 1:1 + zc, 1:1 + D]
                    f = FF[:, w, :]
                    # lo = (p > PS) & (p < 1.5 PS); hi = (p > 12.65)
                    ts(Aa, p, PS, None, A.is_gt)
                    ts(G, p, BLO_HI, None, A.is_lt)
                    tt(Aa, Aa, G, A.mult)            # lo
                    ts(G, p, BHI_TH, None, A.is_gt)  # hi
                    # fb = KN*lo*(1.5PS - p) - KN*hi*(p - 12.65) - ETA*v*(lo+hi)
                    ts(T, p, -KN, KN * BLO_HI, A.mult, A.add)
                    tt(T, T, Aa, A.mult)             # spring lo
                    ts(C, p, -KN, KN * BHI_TH, A.mult, A.add)
                    tt(C, C, G, A.mult)              # spring hi (already -KN*(p-th))
                    tt(T, T, C, A.add)
                    tt(Aa, Aa, G, A.add)             # lo + hi
                    tt(P, vv, Aa, A.mult)
                    ts(P, P, -ETA, None, A.mult)
                    tt(T, T, P, A.add)               # fb complete
                    # update
                    tt(S, T, f, A.subtract)          # fb - f
                    if grav != 0.0:
                        ts(S, S, 1.0, grav, A.mult, A.add)
                    tt(S, S, mkw, A.mult)
                    ts(S, S, DT / PM, None, A.mult)
                    vn = VN[comp][:, w, 1:1 + D]
                    tt(vn, vv, S, A.add)

            # ================= PHASE 0 (layer n = 0) ===================
            p0_chunks = [(1, 6), (7, 6), (13, 6)]  # v0new local planes [1,19)
            for (w0, zc) in p0_chunks:
                zlo, zhi = w0 - 1, w0 + zc + 1
                # n-side: layer 0 aligned
                for f in range(6):
                    load(nt[f][:, 0:zc + 2, :], ext[FIELD_NAMES[f]][0, zlo:zhi, 1:1 + D, :])
                load(mk[:, 0:zc, :], ext["mk"][0, w0:w0 + zc, 1:1 + D, :])
                nc.gpsimd.memset(FX[:, 0:zc, :], 0.0)
                nc.gpsimd.memset(FY[:, 0:zc, :], 0.0)
                nc.gpsimd.memset(FZ[:, 0:zc, :], 0.0)

                nsrc = [nt[f] for f in range(6)]
                # --- m = 0 (same layer): var0 tiles are the nt tiles
                for f in range(6):
                    for v in (-1, 1):
                        load(mt[(f, v)][:, 0:zc + 2, :],
                             ext[FIELD_NAMES[f]][0, zlo:zhi, 1 + v:1 + v + D, :])
                msrc0 = [
                    (lambda f_: (lambda v: nt[f_] if v == 0 else mt[(f_, v)]))(f)
                    for f in range(6)
                ]
                for sh in SHIFTS:
                    if sh == (0, 0, 0):
                        continue
                    emit_combo(zc, sh, nsrc, msrc0)
                # --- m = 1 (other layer)
                for f in range(6):
                    for v in (-1, 0, 1):
                        load(mt[(f, v)][:, 0:zc + 2, :],
                             ext[FIELD_NAMES[f]][1, zlo:zhi, 1 + v:1 + v + D, :])
                msrc1 = [(lambda f_: (lambda v: mt[(f_, v)]))(f) for f in range(6)]
                for sh in SHIFTS:
                    emit_combo(zc, sh, nsrc, msrc1)

                for i in range(3):
                    nc.gpsimd.memset(VN[i][:, 0:zc, :], 0.0)
                boundary_and_update(zc, nsrc, GRAV)
                # write v0new chunk to scratch (planes j = local-1)
                for i in range(3):
                    nc.sync.dma_start(
                        scr[i, w0 - 1:w0 - 1 + zc, 1:1 + D, :]
                        .rearrange("z y x -> y z x"),
                        VN[i][:, 0:zc, :])
                # write the output sub-range (local planes [2,18))
                olo = max(w0, 2)
                ohi = min(w0 + zc, 2 + ZP)
                if ohi > olo:
                    for i in range(3):
                        nc.sync.dma_start(
                            out_ext[i, olo - 2:ohi - 2, :, :]
                            .rearrange("z y x -> y z x"),
                            VN[i][:, olo - w0:ohi - w0, 1:1 + D])

            # ================= PHASE 1 (layer n = 1) ===================
            p1_chunks = [(2, 6), (8, 6), (14, 4)]  # out planes local [2,18)
            for (w0, zc) in p1_chunks:
                zlo, zhi = w0 - 1, w0 + zc + 1
                for f in range(6):
                    load(nt[f][:, 0:zc + 2, :], ext[FIELD_NAMES[f]][1, zlo:zhi, 1:1 + D, :])
                load(mk[:, 0:zc, :], ext["mk"][1, w0:w0 + zc, 1:1 + D, :])
                nc.gpsimd.memset(FX[:, 0:zc, :], 0.0)
                nc.gpsimd.memset(FY[:, 0:zc, :], 0.0)
                nc.gpsimd.memset(FZ[:, 0:zc, :], 0.0)

                nsrc = [nt[f] for f in range(6)]
                # --- m = 1 (same layer)
                for f in range(6):
                    for v in (-1, 1):
                        load(mt[(f, v)][:, 0:zc + 2, :],
                             ext[FIELD_NAMES[f]][1, zlo:zhi, 1 + v:1 + v + D, :])
                msrc1 = [
                    (lambda f_: (lambda v: nt[f_] if v == 0 else mt[(f_, v)]))(f)
                    for f in range(6)
                ]
                for sh in SHIFTS:
                    if sh == (0, 0, 0):
                        continue
                    emit_combo(zc, sh, nsrc, msrc1)
                # --- m = 0: positions from inputs, velocities from scratch
                for f in range(3):
                    for v in (-1, 0, 1):
                        load(mt[(f, v)][:, 0:zc + 2, :],
                             ext[FIELD_NAMES[f]][0, zlo:zhi, 1 + v:1 + v + D, :])
                for f in range(3):
                    for v in (-1, 0, 1):
                        load(mt[(3 + f, v)][:, 0:zc + 2, :],
                             scr[f, w0 - 2:w0 + zc, 1 + v:1 + v + D, :])
                msrc0 = [(lambda f_: (lambda v: mt[(f_, v)]))(f) for f in range(6)]
                for sh in SHIFTS:
                    emit_combo(zc, sh, nsrc, msrc0)

                for i in range(3):
                    nc.gpsimd.memset(VN[i][:, 0:zc, :], 0.0)
                boundary_and_update(zc, nsrc, GRAV)
                for i in range(3):
                    nc.sync.dma_start(
                        out_ext[3 + i, w0 - 2:w0 - 2 + zc, :, :]
                        .rearrange("z y x -> y z x"),
                        VN[i][:, 0:zc, 1:1 + D])

    nc.compile()
    return nc


def _get_compiled():
    global _compiled
    if _compiled is None:
        _compiled = _build()
    return _compiled


def _pad_field(a, val):
    # (2,1,1,D,D,D) -> (2, D+4, NY, NX)
    a = np.ascontiguousarray(a.reshape(2, D, D, D), dtype=np.float32)
    return np.pad(a, ((0, 0), (2, 2), (1, 1), (1, 1)), constant_values=val)


def kernel(x_grid, y_grid, z_grid, vx_grid, vy_grid, vz_grid, mask):
    from concourse.bass_utils import run_bass_kernel_spmd

    nc = _get_compiled()

    padded = {
        "xg": _pad_field(x_grid, SENT),
        "yg": _pad_field(y_grid, SENT),
        "zg": _pad_field(z_grid, SENT),
        "vx": _pad_field(vx_grid, 0.0),
        "vy": _pad_field(vy_grid, 0.0),
        "vz": _pad_field(vz_grid, 0.0),
        "mk": _pad_field(mask, 0.0),
    }
    in_maps = []
    for c in range(NCORES):
        z0 = ZP * c
        in_maps.append({k: np.ascontiguousarray(v[:, z0:z0 + NZIN])
                        for k, v in padded.items()})

    res = run_bass_kernel_spmd(nc, in_maps, core_ids=list(range(NCORES)))

    out = np.empty((3, 2, 1, 1, D, D, D), np.float32)
    for c in range(NCORES):
        o = res.results[c]["out"]  # (6, ZP, D, D)
        z0 = ZP * c
        for comp in range(3):
            out[comp, 0, 0, 0, z0:z0 + ZP] = o[comp]
            out[comp, 1, 0, 0, z0:z0 + ZP] = o[3 + comp]
    return out


# revision 9
# speedup vs baseline: 1.0070x; 1.0033x over previous
"""AI4DEM contact-force kernel for 8 TRN2 NeuronCores.

Physics (from the reference): two particle layers on a 128^3 grid; for each
layer n, accumulate spring-damper contact forces from both layers over a
5x5x5 neighborhood of rolls, then integrate velocities.  Because
cell_size == particle_size == 0.1 and particle jitter < cell_size, any
offset with |shift| >= 2 in some axis can never produce a contact
(dist > PS provably), so the 125-point stencil reduces exactly to 3x3x3.
Roll wrap-around contributions are likewise provably zero (positions
differ by ~12.7), so the stencil is a pure local halo-1 stencil with
far-value sentinels at the global boundary.

Distribution: shard z (first spatial axis) across the 8 cores, 16 planes
each.  Layer-1 forces depend on layer-0's *updated* velocities, so each
core takes a halo of 2 input planes per side (inputs host-padded with
sentinel planes so all cores run an identical program) and no inter-core
communication is needed at all.

Layout on core: partition dim = y (128 rows), free dims = (z-chunk, x).
y-shifted stencil reads are materialized as 3 y-offset DMA loads from the
host-padded (y=130) arrays; z and x shifts are free-dim window offsets.

Precision split: the geometry path (position deltas, dist^2, contact gate,
spring term) is fp32 so the contact gate agrees with the reference to ~1
ulp; the damping path (velocity deltas, relative-velocity dot) and the
force direction products run in bf16 (DVE 2x mode), with accumulation in
fp32.  1/dist and ETA/dist^2 come from Exp(-0.5*Ln(s)) / Exp(-Ln(s)+lnETA)
on the Scalar engine (single activation-table set; DVE reciprocal is ~6x
slower).  Element-wise work is split across DVE / Scalar / GpSimd to keep
all three engines busy.
"""

import math
import sys

import numpy as np

sys.path.insert(0, "/opt/trn_rl_repo")

D = 128
CELL = 0.1
PS = 0.1
KN = 6.0e6
_ALPHA = -math.log(0.5) / math.pi
_GAMMA = _ALPHA / math.sqrt(_ALPHA**2 + 1.0)
PM = 4.0 / 3.0 * 3.1415 * CELL**3 * 2700.0
ETA = 2.0 * _GAMMA * math.sqrt(KN * PM)
DT = 1e-4
SENT = 1.0e3      # far-value sentinel for positions at global boundaries
NCORES = 8
ZP = D // NCORES  # 16 output planes per core
NZIN = ZP + 4     # input planes per core (halo 2 each side)
NY = D + 2        # host-padded y extent
NX = D + 2        # host-padded x extent

BLO_HI = 1.5 * PS
BHI_TH = D * CELL - 0.5 * PS - CELL
GRAV = -9.8 * PM

ZC_MAX = 6
ZCP2 = ZC_MAX + 2

SHIFTS = [(a, b, c) for a in (-1, 0, 1) for b in (-1, 0, 1) for c in (-1, 0, 1)]

POS_NAMES = ["xg", "yg", "zg"]
VEL_NAMES = ["vx", "vy", "vz"]

_compiled = None


def _build():
    from contextlib import ExitStack
    from concourse import bacc, tile, mybir

    f32 = mybir.dt.float32
    bf16 = mybir.dt.bfloat16
    A = mybir.AluOpType
    AF = mybir.ActivationFunctionType

    nc = bacc.Bacc("TRN2", target_bir_lowering=False, debug=False)

    ext = {}
    for f in POS_NAMES + VEL_NAMES + ["mk"]:
        ext[f] = nc.dram_tensor(f, [2, NZIN, NY, NX], f32, kind="ExternalInput").ap()
    extb = {}
    for f in VEL_NAMES:
        extb[f] = nc.dram_tensor(f + "b", [2, NZIN, NY, NX], bf16,
                                 kind="ExternalInput").ap()
    out_ext = nc.dram_tensor("out", [6, ZP, D, D], f32, kind="ExternalOutput").ap()
    # layer-0 updated velocities (bf16: damping-only consumer) for phase 1
    scr = nc.dram_tensor("v0s", [3, ZP + 2, NY, NX], bf16).ap()

    with tile.TileContext(nc) as tc:
        with ExitStack() as ctx:
            pool = ctx.enter_context(tc.tile_pool(name="sbuf", bufs=1))

            def mktile(name, shape, dtp):
                return pool.tile(shape, dtp, name=name, tag=name)

            # n-side aligned: positions+velocities fp32 (geometry / update)
            nt = [mktile(f"nt_{f}", [D, ZCP2, NX], f32) for f in range(6)]
            # n-side aligned velocities bf16 (damping delta operand)
            ntb = [mktile(f"ntb_{f}", [D, ZCP2, NX], bf16) for f in range(3)]
            # m-side positions fp32, velocities bf16; 3 y-variants each
            mp = {(f, v): mktile(f"mp_{f}_{v}", [D, ZCP2, NX], f32)
                  for f in range(3) for v in (-1, 0, 1)}
            mv = {(f, v): mktile(f"mv_{f}_{v}", [D, ZCP2, NX], bf16)
                  for f in range(3) for v in (-1, 0, 1)}
            mk = mktile("mk", [D, ZC_MAX, NX], f32)

            inter = {}
            for tg in ["dX", "dY", "dZ", "QA", "QB", "QC", "S", "LN",
                       "R", "Aa", "G", "P3"]:
                inter[tg] = mktile(tg, [D, ZC_MAX, D], f32)
            for tg in ["dXb", "dYb", "dZb", "R2", "VA", "VB", "VC",
                       "P1", "P2", "C", "TX", "TY", "TZ"]:
                inter[tg] = mktile(tg, [D, ZC_MAX, D], bf16)
            FX = mktile("FX", [D, ZC_MAX, D], f32)
            FY = mktile("FY", [D, ZC_MAX, D], f32)
            FZ = mktile("FZ", [D, ZC_MAX, D], f32)
            VN = [mktile(f"VN{i}", [D, ZC_MAX, NX], f32) for i in range(3)]
            VNB = [mktile(f"VNB{i}", [D, ZC_MAX, NX], bf16) for i in range(3)]

            b_eps = mktile("b_eps", [D, 1], f32)
            b_lneta = mktile("b_lneta", [D, 1], f32)
            b_kn = mktile("b_kn", [D, 1], f32)
            zt = mktile("zt", [D, NX], bf16)
            nc.vector.memset(b_eps[:], 1e-8)
            nc.vector.memset(b_lneta[:], math.log(ETA))
            nc.vector.memset(b_kn[:], KN)
            nc.vector.memset(zt[:], 0.0)

            for f in range(3):
                nc.sync.dma_start(scr[f, :, 0, :], zt[0:ZP + 2, :])
                nc.sync.dma_start(scr[f, :, NY - 1, :], zt[0:ZP + 2, :])

            def load(tile_t, src_ap):
                nc.sync.dma_start(tile_t, src_ap.rearrange("z y x -> y z x"))

            def emit_combo(zc, sh, npos, nvelb, mpos, mvelb):
                shz, shy, shx = sh
                v = -shy
                z0, x0 = 1 - shz, 1 - shx
                msl = (slice(None), slice(z0, z0 + zc), slice(x0, x0 + D))
                nsl = (slice(None), slice(1, 1 + zc), slice(1, 1 + D))
                w = slice(0, zc)
                I = {k: t[:, w, :] for k, t in inter.items()}
                fx, fy, fz = FX[:, w, :], FY[:, w, :], FZ[:, w, :]

                tt = nc.vector.tensor_tensor
                ts = nc.vector.tensor_scalar
                gp = nc.gpsimd.tensor_tensor
                act = nc.scalar.activation

                # geometry: fp32
                tt(I["dX"], npos[0][nsl], mpos[0](v)[msl], A.subtract)
                tt(I["dY"], npos[1][nsl], mpos[1](v)[msl], A.subtract)
                tt(I["dZ"], npos[2][nsl], mpos[2](v)[msl], A.subtract)
                act(I["QA"], I["dX"], AF.Square)
                act(I["QB"], I["dY"], AF.Square)
                act(I["QC"], I["dZ"], AF.Square)
                act(I["dXb"], I["dX"], AF.Copy)
                act(I["dYb"], I["dY"], AF.Copy)
                act(I["dZb"], I["dZ"], AF.Copy)
                gp(I["S"], I["QA"], I["QB"], A.add)
                gp(I["S"], I["S"], I["QC"], A.add)
                act(I["LN"], I["S"], AF.Ln, bias=b_eps[:], scale=1.0)
                act(I["R"], I["LN"], AF.Exp, bias=0.0, scale=-0.5)
                act(I["R2"], I["LN"], AF.Exp, bias=b_lneta[:], scale=-1.0)
                # spring term: KN - KN*PS*r  (ACT affine)
                act(I["Aa"], I["R"], AF.Identity, bias=b_kn[:], scale=-KN * PS)
                # contact gate on s directly: [dist < PS] == [s < PS^2]
                ts(I["G"], I["S"], PS * PS, None, A.is_lt)
                # damping: bf16
                gp(I["VA"], nvelb[0][nsl], mvelb[0](v)[msl], A.subtract)
                gp(I["VB"], nvelb[1][nsl], mvelb[1](v)[msl], A.subtract)
                gp(I["VC"], nvelb[2][nsl], mvelb[2](v)[msl], A.subtract)
                tt(I["P1"], I["VA"], I["dXb"], A.mult)
                tt(I["P2"], I["VB"], I["dYb"], A.mult)
                tt(I["TX"], I["VC"], I["dZb"], A.mult)
                gp(I["P1"], I["P1"], I["P2"], A.add)
                gp(I["P1"], I["P1"], I["TX"], A.add)
                tt(I["P2"], I["P1"], I["R2"], A.mult)   # ETA * dvn_raw / s
                tt(I["P3"], I["Aa"], I["P2"], A.add)
                tt(I["C"], I["P3"], I["G"], A.mult)     # gated coef (bf16 out)
                tt(I["TX"], I["C"], I["dXb"], A.mult)
                tt(I["TY"], I["C"], I["dYb"], A.mult)
                tt(I["TZ"], I["C"], I["dZb"], A.mult)
                gp(fx, fx, I["TX"], A.add)
                gp(fy, fy, I["TY"], A.add)
                gp(fz, fz, I["TZ"], A.add)

            def boundary_and_update(zc, nsrc, write_bf16):
                tt = nc.vector.tensor_tensor
                ts = nc.vector.tensor_scalar
                w = slice(0, zc)
                mkw = mk[:, w, 1:1 + D]
                Aa = inter["Aa"][:, w, :]
                G = inter["G"][:, w, :]
                P = inter["P3"][:, w, :]
                T = inter["dX"][:, w, :]
                C = inter["dY"][:, w, :]
                S = inter["S"][:, w, :]
                for comp, (FF, grav) in enumerate(
                        [(FX, 0.0), (FY, 0.0), (FZ, GRAV)]):
                    p = nsrc[comp][:, 1:1 + zc, 1:1 + D]
                    vv = nsrc[3 + comp][:, 1:1 + zc, 1:1 + D]
                    f = FF[:, w, :]
                    ts(Aa, p, PS, None, A.is_gt)
                    ts(G, p, BLO_HI, None, A.is_lt)
                    tt(Aa, Aa, G, A.mult)            # lo
                    ts(G, p, BHI_TH, None, A.is_gt)  # hi
                    ts(T, p, -KN, KN * BLO_HI, A.mult, A.add)
                    tt(T, T, Aa, A.mult)
                    ts(C, p, -KN, KN * BHI_TH, A.mult, A.add)
                    tt(C, C, G, A.mult)
                    tt(T, T, C, A.add)
                    tt(Aa, Aa, G, A.add)             # lo + hi
                    tt(P, vv, Aa, A.mult)
                    ts(P, P, -ETA, None, A.mult)
                    tt(T, T, P, A.add)               # fb
                    tt(S, T, f, A.subtract)
                    if grav != 0.0:
                        ts(S, S, 1.0, grav, A.mult, A.add)
                    tt(S, S, mkw, A.mult)
                    ts(S, S, DT / PM, None, A.mult)
                    vn = VN[comp][:, w, 1:1 + D]
                    tt(vn, vv, S, A.add)
                    if write_bf16:
                        vnb = VNB[comp][:, w, 1:1 + D]
                        tt(vnb, vv, S, A.add)

            def phase(n, chunks, m_list, vel_src):
                """vel_src[m] -> ('ext', layer) or ('scr',) for bf16 vel loads"""
                for (w0, zc) in chunks:
                    zlo, zhi = w0 - 1, w0 + zc + 1
                    for f in range(3):
                        load(nt[f][:, 0:zc + 2, :],
                             ext[POS_NAMES[f]][n, zlo:zhi, 1:1 + D, :])
                        load(nt[3 + f][:, 0:zc + 2, :],
                             ext[VEL_NAMES[f]][n, zlo:zhi, 1:1 + D, :])
                        load(ntb[f][:, 0:zc + 2, :],
                             extb[VEL_NAMES[f]][n, zlo:zhi, 1:1 + D, :])
                    load(mk[:, 0:zc, :], ext["mk"][n, w0:w0 + zc, 1:1 + D, :])
                    nc.gpsimd.memset(FX[:, 0:zc, :], 0.0)
                    nc.gpsimd.memset(FY[:, 0:zc, :], 0.0)
                    nc.gpsimd.memset(FZ[:, 0:zc, :], 0.0)

                    npos = [nt[0], nt[1], nt[2]]
                    nvelb = ntb
                    for m in m_list:
                        same = (m == n)
                        for f in range(3):
                            vs = ((-1, 1) if same else (-1, 0, 1))
                            for v in vs:
                                load(mp[(f, v)][:, 0:zc + 2, :],
                                     ext[POS_NAMES[f]][m, zlo:zhi, 1 + v:1 + v + D, :])
                            src = vel_src[m]
                            for v in (-1, 0, 1):
                                if src[0] == "ext":
                                    load(mv[(f, v)][:, 0:zc + 2, :],
                                         extb[VEL_NAMES[f]][m, zlo:zhi,
                                                            1 + v:1 + v + D, :])
                                else:
                                    load(mv[(f, v)][:, 0:zc + 2, :],
                                         scr[f, w0 - 2:w0 + zc,
                                             1 + v:1 + v + D, :])
                        if same:
                            mpos = [(lambda f_: (lambda v: nt[f_] if v == 0
                                                 else mp[(f_, v)]))(f)
                                    for f in range(3)]
                        else:
                            mpos = [(lambda f_: (lambda v: mp[(f_, v)]))(f)
                                    for f in range(3)]
                        mvelb = [(lambda f_: (lambda v: mv[(f_, v)]))(f)
                                 for f in range(3)]
                        for sh in SHIFTS:
                            if same and sh == (0, 0, 0):
                                continue
                            emit_combo(zc, sh, npos, nvelb, mpos, mvelb)

                    for i in range(3):
                        nc.gpsimd.memset(VN[i][:, 0:zc, :], 0.0)
                        if n == 0:
                            nc.gpsimd.memset(VNB[i][:, 0:zc, :], 0.0)
                    boundary_and_update(zc, nt, write_bf16=(n == 0))
                    if n == 0:
                        for i in range(3):
                            nc.sync.dma_start(
                                scr[i, w0 - 1:w0 - 1 + zc, 1:1 + D, :]
                                .rearrange("z y x -> y z x"),
                                VNB[i][:, 0:zc, :])
                        olo, ohi = max(w0, 2), min(w0 + zc, 2 + ZP)
                        if ohi > olo:
                            for i in range(3):
                                nc.sync.dma_start(
                                    out_ext[i, olo - 2:ohi - 2, :, :]
                                    .rearrange("z y x -> y z x"),
                                    VN[i][:, olo - w0:ohi - w0, 1:1 + D])
                    else:
                        for i in range(3):
                            nc.sync.dma_start(
                                out_ext[3 + i, w0 - 2:w0 - 2 + zc, :, :]
                                .rearrange("z y x -> y z x"),
                                VN[i][:, 0:zc, 1:1 + D])

            # phase 0: n=0; m=0 (same layer) then m=1, vel from ext bf16
            phase(0, [(1, 6), (7, 6), (13, 6)], [0, 1],
                  {0: ("ext",), 1: ("ext",)})
            # phase 1: n=1; m=1 (same layer) then m=0 with vel from scratch
            phase(1, [(2, 6), (8, 6), (14, 4)], [1, 0],
                  {1: ("ext",), 0: ("scr",)})

    nc.compile()
    return nc


def _get_compiled():
    global _compiled
    if _compiled is None:
        _compiled = _build()
    return _compiled


def _pad_field(a, val):
    a = np.ascontiguousarray(a.reshape(2, D, D, D), dtype=np.float32)
    return np.pad(a, ((0, 0), (2, 2), (1, 1), (1, 1)), constant_values=val)


def kernel(x_grid, y_grid, z_grid, vx_grid, vy_grid, vz_grid, mask):
    import ml_dtypes
    from concourse.bass_utils import run_bass_kernel_spmd

    nc = _get_compiled()

    padded = {
        "xg": _pad_field(x_grid, SENT),
        "yg": _pad_field(y_grid, SENT),
        "zg": _pad_field(z_grid, SENT),
        "vx": _pad_field(vx_grid, 0.0),
        "vy": _pad_field(vy_grid, 0.0),
        "vz": _pad_field(vz_grid, 0.0),
        "mk": _pad_field(mask, 0.0),
    }
    for f in VEL_NAMES:
        padded[f + "b"] = padded[f].astype(ml_dtypes.bfloat16)

    in_maps = []
    for c in range(NCORES):
        z0 = ZP * c
        in_maps.append({k: np.ascontiguousarray(v[:, z0:z0 + NZIN])
                        for k, v in padded.items()})

    res = run_bass_kernel_spmd(nc, in_maps, core_ids=list(range(NCORES)))

    out = np.empty((3, 2, 1, 1, D, D, D), np.float32)
    for c in range(NCORES):
        o = res.results[c]["out"]
        z0 = ZP * c
        for comp in range(3):
            out[comp, 0, 0, 0, z0:z0 + ZP] = o[comp]
            out[comp, 1, 0, 0, z0:z0 + ZP] = o[3 + comp]
    return out


# revision 10
# speedup vs baseline: 1.5157x; 1.5052x over previous
"""AI4DEM contact-force kernel for 8 TRN2 NeuronCores.

Physics (from the reference): two particle layers on a 128^3 grid; for each
layer n, accumulate spring-damper contact forces from both layers over a
5x5x5 neighborhood of rolls, then integrate velocities.  Because
cell_size == particle_size == 0.1 and particle jitter < cell_size, any
offset with |shift| >= 2 in some axis can never produce a contact
(dist > PS provably), so the 125-point stencil reduces exactly to 3x3x3.
Roll wrap-around contributions are likewise provably zero (positions
differ by ~12.7), so the stencil is a pure local halo-1 stencil with
far-value sentinels at the global boundary.

Distribution: shard z (first spatial axis) across the 8 cores, 16 planes
each.  Layer-1 forces depend on layer-0's *updated* velocities, so each
core takes a halo of 2 input planes per side (inputs host-padded with
sentinel planes so all cores run an identical program) and no inter-core
communication is needed at all.

Layout on core: partition dim = y (128 rows), free dims = (z-chunk, x).
y-shifted stencil reads are materialized as 3 y-offset DMA loads from the
host-padded (y=130) arrays; z and x shifts are free-dim window offsets.

Precision split: the geometry path (position deltas, dist^2, contact gate,
spring term) is fp32 so the contact gate agrees with the reference to ~1
ulp; the damping path (velocity deltas, relative-velocity dot) and the
force direction products run in bf16 (DVE 2x mode), with accumulation in
fp32.  1/dist and ETA/dist^2 come from Exp(-0.5*Ln(s)) / Exp(-Ln(s)+lnETA)
on the Scalar engine (single activation-table set; DVE reciprocal is ~6x
slower).  Element-wise work is split across DVE / Scalar / GpSimd to keep
all three engines busy.
"""

import math
import sys

import numpy as np

sys.path.insert(0, "/opt/trn_rl_repo")

D = 128
CELL = 0.1
PS = 0.1
KN = 6.0e6
_ALPHA = -math.log(0.5) / math.pi
_GAMMA = _ALPHA / math.sqrt(_ALPHA**2 + 1.0)
PM = 4.0 / 3.0 * 3.1415 * CELL**3 * 2700.0
ETA = 2.0 * _GAMMA * math.sqrt(KN * PM)
DT = 1e-4
SENT = 1.0e3      # far-value sentinel for positions at global boundaries
NCORES = 8
ZP = D // NCORES  # 16 output planes per core
NZIN = ZP + 4     # input planes per core (halo 2 each side)
NY = D + 2        # host-padded y extent
NX = D + 2        # host-padded x extent

BLO_HI = 1.5 * PS
BHI_TH = D * CELL - 0.5 * PS - CELL
GRAV = -9.8 * PM

ZC_MAX = 6
ZCP2 = ZC_MAX + 2

SHIFTS = [(a, b, c) for a in (-1, 0, 1) for b in (-1, 0, 1) for c in (-1, 0, 1)]

POS_NAMES = ["xg", "yg", "zg"]
VEL_NAMES = ["vx", "vy", "vz"]

_compiled = None


def _build():
    from contextlib import ExitStack
    from concourse import bacc, tile, mybir

    f32 = mybir.dt.float32
    bf16 = mybir.dt.bfloat16
    A = mybir.AluOpType
    AF = mybir.ActivationFunctionType

    nc = bacc.Bacc("TRN2", target_bir_lowering=False, debug=False)

    ext = {}
    for f in POS_NAMES + VEL_NAMES + ["mk"]:
        ext[f] = nc.dram_tensor(f, [2, NZIN, NY, NX], f32, kind="ExternalInput").ap()
    extb = {}
    for f in VEL_NAMES:
        extb[f] = nc.dram_tensor(f + "b", [2, NZIN, NY, NX], bf16,
                                 kind="ExternalInput").ap()
    out_ext = nc.dram_tensor("out", [6, ZP, D, D], f32, kind="ExternalOutput").ap()
    # layer-0 updated velocities (bf16: damping-only consumer) for phase 1
    scr = nc.dram_tensor("v0s", [3, ZP + 2, NY, NX], bf16).ap()

    with tile.TileContext(nc) as tc:
        with ExitStack() as ctx:
            pool = ctx.enter_context(tc.tile_pool(name="sbuf", bufs=1))

            def mktile(name, shape, dtp):
                return pool.tile(shape, dtp, name=name, tag=name)

            # n-side aligned: positions+velocities fp32 (geometry / update)
            nt = [mktile(f"nt_{f}", [D, ZCP2, NX], f32) for f in range(6)]
            # n-side aligned velocities bf16 (damping delta operand)
            ntb = [mktile(f"ntb_{f}", [D, ZCP2, NX], bf16) for f in range(3)]
            # m-side positions fp32, velocities bf16; 3 y-variants each
            mp = {(f, v): mktile(f"mp_{f}_{v}", [D, ZCP2, NX], f32)
                  for f in range(3) for v in (-1, 0, 1)}
            mv = {(f, v): mktile(f"mv_{f}_{v}", [D, ZCP2, NX], bf16)
                  for f in range(3) for v in (-1, 0, 1)}
            mk = mktile("mk", [D, ZC_MAX, NX], f32)

            inter = {}
            for tg in ["dX", "dY", "dZ", "QA", "QB", "QC", "S", "LN",
                       "R", "Aa", "G", "P3"]:
                inter[tg] = mktile(tg, [D, ZC_MAX, D], f32)
            for tg in ["dXb", "dYb", "dZb", "R2", "VA", "VB", "VC",
                       "P1", "P2", "C", "TX", "TY", "TZ"]:
                inter[tg] = mktile(tg, [D, ZC_MAX, D], bf16)
            FX = mktile("FX", [D, ZC_MAX, D], f32)
            FY = mktile("FY", [D, ZC_MAX, D], f32)
            FZ = mktile("FZ", [D, ZC_MAX, D], f32)
            VN = [mktile(f"VN{i}", [D, ZC_MAX, NX], f32) for i in range(3)]
            VNB = [mktile(f"VNB{i}", [D, ZC_MAX, NX], bf16) for i in range(3)]

            b_eps = mktile("b_eps", [D, 1], f32)
            b_lneta = mktile("b_lneta", [D, 1], f32)
            b_kn = mktile("b_kn", [D, 1], f32)
            zt = mktile("zt", [D, NX], bf16)
            nc.vector.memset(b_eps[:], 1e-8)
            nc.vector.memset(b_lneta[:], math.log(ETA))
            nc.vector.memset(b_kn[:], KN)
            nc.vector.memset(zt[:], 0.0)

            for f in range(3):
                nc.sync.dma_start(scr[f, :, 0, :], zt[0:ZP + 2, :])
                nc.sync.dma_start(scr[f, :, NY - 1, :], zt[0:ZP + 2, :])

            def load(tile_t, src_ap):
                nc.sync.dma_start(tile_t, src_ap.rearrange("z y x -> y z x"))

            def emit_combo(zc, sh, npos, nvelb, mpos, mvelb):
                shz, shy, shx = sh
                v = -shy
                z0, x0 = 1 - shz, 1 - shx
                msl = (slice(None), slice(z0, z0 + zc), slice(x0, x0 + D))
                nsl = (slice(None), slice(1, 1 + zc), slice(1, 1 + D))
                w = slice(0, zc)
                I = {k: t[:, w, :] for k, t in inter.items()}
                fx, fy, fz = FX[:, w, :], FY[:, w, :], FZ[:, w, :]

                tt = nc.vector.tensor_tensor
                ts = nc.vector.tensor_scalar
                gp = nc.gpsimd.tensor_tensor
                act = nc.scalar.activation

                # geometry: fp32
                tt(I["dX"], npos[0][nsl], mpos[0](v)[msl], A.subtract)
                tt(I["dY"], npos[1][nsl], mpos[1](v)[msl], A.subtract)
                tt(I["dZ"], npos[2][nsl], mpos[2](v)[msl], A.subtract)
                act(I["QA"], I["dX"], AF.Square)
                act(I["QB"], I["dY"], AF.Square)
                act(I["QC"], I["dZ"], AF.Square)
                act(I["dXb"], I["dX"], AF.Copy)
                act(I["dYb"], I["dY"], AF.Copy)
                act(I["dZb"], I["dZ"], AF.Copy)
                tt(I["S"], I["QA"], I["QB"], A.add)
                tt(I["S"], I["S"], I["QC"], A.add)
                act(I["LN"], I["S"], AF.Ln, bias=b_eps[:], scale=1.0)
                act(I["R"], I["LN"], AF.Exp, bias=0.0, scale=-0.5)
                act(I["R2"], I["LN"], AF.Exp, bias=b_lneta[:], scale=-1.0)
                # spring term: KN - KN*PS*r  (ACT affine)
                act(I["Aa"], I["R"], AF.Identity, bias=b_kn[:], scale=-KN * PS)
                # contact gate on s directly: [dist < PS] == [s < PS^2]
                ts(I["G"], I["S"], PS * PS, None, A.is_lt)
                # damping: bf16
                tt(I["VA"], nvelb[0][nsl], mvelb[0](v)[msl], A.subtract)
                tt(I["VB"], nvelb[1][nsl], mvelb[1](v)[msl], A.subtract)
                tt(I["VC"], nvelb[2][nsl], mvelb[2](v)[msl], A.subtract)
                tt(I["P1"], I["VA"], I["dXb"], A.mult)
                tt(I["P2"], I["VB"], I["dYb"], A.mult)
                tt(I["TX"], I["VC"], I["dZb"], A.mult)
                tt(I["P1"], I["P1"], I["P2"], A.add)
                tt(I["P1"], I["P1"], I["TX"], A.add)
                tt(I["P2"], I["P1"], I["R2"], A.mult)   # ETA * dvn_raw / s
                tt(I["P3"], I["Aa"], I["P2"], A.add)
                tt(I["C"], I["P3"], I["G"], A.mult)     # gated coef (bf16 out)
                tt(I["TX"], I["C"], I["dXb"], A.mult)
                tt(I["TY"], I["C"], I["dYb"], A.mult)
                tt(I["TZ"], I["C"], I["dZb"], A.mult)
                tt(fx, fx, I["TX"], A.add)
                tt(fy, fy, I["TY"], A.add)
                tt(fz, fz, I["TZ"], A.add)

            def boundary_and_update(zc, nsrc, write_bf16):
                tt = nc.vector.tensor_tensor
                ts = nc.vector.tensor_scalar
                w = slice(0, zc)
                mkw = mk[:, w, 1:1 + D]
                Aa = inter["Aa"][:, w, :]
                G = inter["G"][:, w, :]
                P = inter["P3"][:, w, :]
                T = inter["dX"][:, w, :]
                C = inter["dY"][:, w, :]
                S = inter["S"][:, w, :]
                for comp, (FF, grav) in enumerate(
                        [(FX, 0.0), (FY, 0.0), (FZ, GRAV)]):
                    p = nsrc[comp][:, 1:1 + zc, 1:1 + D]
                    vv = nsrc[3 + comp][:, 1:1 + zc, 1:1 + D]
                    f = FF[:, w, :]
                    ts(Aa, p, PS, None, A.is_gt)
                    ts(G, p, BLO_HI, None, A.is_lt)
                    tt(Aa, Aa, G, A.mult)            # lo
                    ts(G, p, BHI_TH, None, A.is_gt)  # hi
                    ts(T, p, -KN, KN * BLO_HI, A.mult, A.add)
                    tt(T, T, Aa, A.mult)
                    ts(C, p, -KN, KN * BHI_TH, A.mult, A.add)
                    tt(C, C, G, A.mult)
                    tt(T, T, C, A.add)
                    tt(Aa, Aa, G, A.add)             # lo + hi
                    tt(P, vv, Aa, A.mult)
                    ts(P, P, -ETA, None, A.mult)
                    tt(T, T, P, A.add)               # fb
                    tt(S, T, f, A.subtract)
                    if grav != 0.0:
                        ts(S, S, 1.0, grav, A.mult, A.add)
                    tt(S, S, mkw, A.mult)
                    ts(S, S, DT / PM, None, A.mult)
                    vn = VN[comp][:, w, 1:1 + D]
                    tt(vn, vv, S, A.add)
                    if write_bf16:
                        vnb = VNB[comp][:, w, 1:1 + D]
                        tt(vnb, vv, S, A.add)

            def phase(n, chunks, m_list, vel_src):
                """vel_src[m] -> ('ext', layer) or ('scr',) for bf16 vel loads"""
                for (w0, zc) in chunks:
                    zlo, zhi = w0 - 1, w0 + zc + 1
                    for f in range(3):
                        load(nt[f][:, 0:zc + 2, :],
                             ext[POS_NAMES[f]][n, zlo:zhi, 1:1 + D, :])
                        load(nt[3 + f][:, 0:zc + 2, :],
                             ext[VEL_NAMES[f]][n, zlo:zhi, 1:1 + D, :])
                        load(ntb[f][:, 0:zc + 2, :],
                             extb[VEL_NAMES[f]][n, zlo:zhi, 1:1 + D, :])
                    load(mk[:, 0:zc, :], ext["mk"][n, w0:w0 + zc, 1:1 + D, :])
                    nc.gpsimd.memset(FX[:, 0:zc, :], 0.0)
                    nc.gpsimd.memset(FY[:, 0:zc, :], 0.0)
                    nc.gpsimd.memset(FZ[:, 0:zc, :], 0.0)

                    npos = [nt[0], nt[1], nt[2]]
                    nvelb = ntb
                    for m in m_list:
                        same = (m == n)
                        for f in range(3):
                            vs = ((-1, 1) if same else (-1, 0, 1))
                            for v in vs:
                                load(mp[(f, v)][:, 0:zc + 2, :],
                                     ext[POS_NAMES[f]][m, zlo:zhi, 1 + v:1 + v + D, :])
                            src = vel_src[m]
                            for v in (-1, 0, 1):
                                if src[0] == "ext":
                                    load(mv[(f, v)][:, 0:zc + 2, :],
                                         extb[VEL_NAMES[f]][m, zlo:zhi,
                                                            1 + v:1 + v + D, :])
                                else:
                                    load(mv[(f, v)][:, 0:zc + 2, :],
                                         scr[f, w0 - 2:w0 + zc,
                                             1 + v:1 + v + D, :])
                        if same:
                            mpos = [(lambda f_: (lambda v: nt[f_] if v == 0
                                                 else mp[(f_, v)]))(f)
                                    for f in range(3)]
                        else:
                            mpos = [(lambda f_: (lambda v: mp[(f_, v)]))(f)
                                    for f in range(3)]
                        mvelb = [(lambda f_: (lambda v: mv[(f_, v)]))(f)
                                 for f in range(3)]
                        for sh in SHIFTS:
                            if same and sh == (0, 0, 0):
                                continue
                            emit_combo(zc, sh, npos, nvelb, mpos, mvelb)

                    for i in range(3):
                        nc.gpsimd.memset(VN[i][:, 0:zc, :], 0.0)
                        if n == 0:
                            nc.gpsimd.memset(VNB[i][:, 0:zc, :], 0.0)
                    boundary_and_update(zc, nt, write_bf16=(n == 0))
                    if n == 0:
                        for i in range(3):
                            nc.sync.dma_start(
                                scr[i, w0 - 1:w0 - 1 + zc, 1:1 + D, :]
                                .rearrange("z y x -> y z x"),
                                VNB[i][:, 0:zc, :])
                        olo, ohi = max(w0, 2), min(w0 + zc, 2 + ZP)
                        if ohi > olo:
                            for i in range(3):
                                nc.sync.dma_start(
                                    out_ext[i, olo - 2:ohi - 2, :, :]
                                    .rearrange("z y x -> y z x"),
                                    VN[i][:, olo - w0:ohi - w0, 1:1 + D])
                    else:
                        for i in range(3):
                            nc.sync.dma_start(
                                out_ext[3 + i, w0 - 2:w0 - 2 + zc, :, :]
                                .rearrange("z y x -> y z x"),
                                VN[i][:, 0:zc, 1:1 + D])

            # phase 0: n=0; m=0 (same layer) then m=1, vel from ext bf16
            phase(0, [(1, 6), (7, 6), (13, 6)], [0, 1],
                  {0: ("ext",), 1: ("ext",)})
            # phase 1: n=1; m=1 (same layer) then m=0 with vel from scratch
            phase(1, [(2, 6), (8, 6), (14, 4)], [1, 0],
                  {1: ("ext",), 0: ("scr",)})

    nc.compile()
    return nc


def _get_compiled():
    global _compiled
    if _compiled is None:
        _compiled = _build()
    return _compiled


def _pad_field(a, val):
    a = np.ascontiguousarray(a.reshape(2, D, D, D), dtype=np.float32)
    return np.pad(a, ((0, 0), (2, 2), (1, 1), (1, 1)), constant_values=val)


def kernel(x_grid, y_grid, z_grid, vx_grid, vy_grid, vz_grid, mask):
    import ml_dtypes
    from concourse.bass_utils import run_bass_kernel_spmd

    nc = _get_compiled()

    padded = {
        "xg": _pad_field(x_grid, SENT),
        "yg": _pad_field(y_grid, SENT),
        "zg": _pad_field(z_grid, SENT),
        "vx": _pad_field(vx_grid, 0.0),
        "vy": _pad_field(vy_grid, 0.0),
        "vz": _pad_field(vz_grid, 0.0),
        "mk": _pad_field(mask, 0.0),
    }
    for f in VEL_NAMES:
        padded[f + "b"] = padded[f].astype(ml_dtypes.bfloat16)

    in_maps = []
    for c in range(NCORES):
        z0 = ZP * c
        in_maps.append({k: np.ascontiguousarray(v[:, z0:z0 + NZIN])
                        for k, v in padded.items()})

    res = run_bass_kernel_spmd(nc, in_maps, core_ids=list(range(NCORES)))

    out = np.empty((3, 2, 1, 1, D, D, D), np.float32)
    for c in range(NCORES):
        o = res.results[c]["out"]
        z0 = ZP * c
        for comp in range(3):
            out[comp, 0, 0, 0, z0:z0 + ZP] = o[comp]
            out[comp, 1, 0, 0, z0:z0 + ZP] = o[3 + comp]
    return out


# revision 11
# speedup vs baseline: 1.8190x; 1.2001x over previous
"""AI4DEM contact-force kernel for 8 TRN2 NeuronCores.

Physics (from the reference): two particle layers on a 128^3 grid; for each
layer n, accumulate spring-damper contact forces from both layers over a
5x5x5 neighborhood of rolls, then integrate velocities.  Because
cell_size == particle_size == 0.1 and particle jitter < cell_size, any
offset with |shift| >= 2 in some axis can never produce a contact
(dist > PS provably), so the 125-point stencil reduces exactly to 3x3x3.
Roll wrap-around contributions are likewise provably zero (positions
differ by ~12.7), so the stencil is a pure local halo-1 stencil with
far-value sentinels at the global boundary.

Distribution: shard z (first spatial axis) across the 8 cores, 16 planes
each.  Layer-1 forces depend on layer-0's *updated* velocities, so each
core takes a halo of 2 input planes per side (inputs host-padded with
sentinel planes so all cores run an identical program) and no inter-core
communication is needed at all.

Layout on core: partition dim = y (128 rows), free dims = (z-chunk, x).
y-shifted stencil reads are materialized as 3 y-offset DMA loads from the
host-padded (y=130) arrays; z and x shifts are free-dim window offsets.

Precision split: the geometry path (position deltas, dist^2, contact gate,
spring term) is fp32 so the contact gate agrees with the reference to ~1
ulp; the damping path (velocity deltas, relative-velocity dot) and the
force direction products run in bf16 (DVE 2x mode), with accumulation in
fp32.  1/dist and ETA/dist^2 come from Exp(-0.5*Ln(s)) / Exp(-Ln(s)+lnETA)
on the Scalar engine (single activation-table set; DVE reciprocal is ~6x
slower).  Element-wise work is split across DVE / Scalar / GpSimd to keep
all three engines busy.
"""

import math
import sys

import numpy as np

sys.path.insert(0, "/opt/trn_rl_repo")

D = 128
CELL = 0.1
PS = 0.1
KN = 6.0e6
_ALPHA = -math.log(0.5) / math.pi
_GAMMA = _ALPHA / math.sqrt(_ALPHA**2 + 1.0)
PM = 4.0 / 3.0 * 3.1415 * CELL**3 * 2700.0
ETA = 2.0 * _GAMMA * math.sqrt(KN * PM)
DT = 1e-4
SENT = 1.0e3      # far-value sentinel for positions at global boundaries
NCORES = 8
ZP = D // NCORES  # 16 output planes per core
NZIN = ZP + 4     # input planes per core (halo 2 each side)
NY = D + 2        # host-padded y extent
NX = D + 2        # host-padded x extent

BLO_HI = 1.5 * PS
BHI_TH = D * CELL - 0.5 * PS - CELL
GRAV = -9.8 * PM

ZC_MAX = 6
ZCP2 = ZC_MAX + 2

SHIFTS = [(a, b, c) for a in (-1, 0, 1) for b in (-1, 0, 1) for c in (-1, 0, 1)]

POS_NAMES = ["xg", "yg", "zg"]
VEL_NAMES = ["vx", "vy", "vz"]

_compiled = None


def _build():
    from contextlib import ExitStack
    from concourse import bacc, tile, mybir

    f32 = mybir.dt.float32
    bf16 = mybir.dt.bfloat16
    A = mybir.AluOpType
    AF = mybir.ActivationFunctionType

    nc = bacc.Bacc("TRN2", target_bir_lowering=False, debug=False)

    # All ACT funcs used here (Square, Copy, Identity, Ln, Exp) live in the
    # "natural_log_exp_and_others" table set, but the default first-match
    # table choice pairs Exp with set 0 and Ln with set 5, inserting ~640
    # table reloads (~2.7us each).  Blank every other set (the cached dict
    # is shared, and set *indices* are positional, so contents must be
    # emptied rather than removed) so one table load serves the kernel.
    from concourse import hw_specs
    tabs = hw_specs.get_activation_tables(nc.m.arch)
    for k in tabs:
        if k != "natural_log_exp_and_others":
            tabs[k] = set()

    ext = {}
    for f in POS_NAMES + VEL_NAMES + ["mk"]:
        ext[f] = nc.dram_tensor(f, [2, NZIN, NY, NX], f32, kind="ExternalInput").ap()
    extb = {}
    for f in VEL_NAMES:
        extb[f] = nc.dram_tensor(f + "b", [2, NZIN, NY, NX], bf16,
                                 kind="ExternalInput").ap()
    out_ext = nc.dram_tensor("out", [6, ZP, D, D], f32, kind="ExternalOutput").ap()
    # layer-0 updated velocities (bf16: damping-only consumer) for phase 1
    scr = nc.dram_tensor("v0s", [3, ZP + 2, NY, NX], bf16).ap()

    with tile.TileContext(nc) as tc:
        with ExitStack() as ctx:
            pool = ctx.enter_context(tc.tile_pool(name="sbuf", bufs=1))

            def mktile(name, shape, dtp):
                return pool.tile(shape, dtp, name=name, tag=name)

            # n-side aligned: positions+velocities fp32 (geometry / update)
            nt = [mktile(f"nt_{f}", [D, ZCP2, NX], f32) for f in range(6)]
            # n-side aligned velocities bf16 (damping delta operand)
            ntb = [mktile(f"ntb_{f}", [D, ZCP2, NX], bf16) for f in range(3)]
            # m-side positions fp32, velocities bf16; 3 y-variants each
            mp = {(f, v): mktile(f"mp_{f}_{v}", [D, ZCP2, NX], f32)
                  for f in range(3) for v in (-1, 0, 1)}
            mv = {(f, v): mktile(f"mv_{f}_{v}", [D, ZCP2, NX], bf16)
                  for f in range(3) for v in (-1, 0, 1)}
            mk = mktile("mk", [D, ZC_MAX, NX], f32)

            inter = {}
            for tg in ["dX", "dY", "dZ", "QA", "QB", "QC", "S", "LN",
                       "R", "Aa", "G", "P3"]:
                inter[tg] = mktile(tg, [D, ZC_MAX, D], f32)
            for tg in ["dXb", "dYb", "dZb", "R2", "VA", "VB", "VC",
                       "P1", "P2", "C", "TX", "TY", "TZ"]:
                inter[tg] = mktile(tg, [D, ZC_MAX, D], bf16)
            FX = mktile("FX", [D, ZC_MAX, D], f32)
            FY = mktile("FY", [D, ZC_MAX, D], f32)
            FZ = mktile("FZ", [D, ZC_MAX, D], f32)
            VN = [mktile(f"VN{i}", [D, ZC_MAX, NX], f32) for i in range(3)]
            VNB = [mktile(f"VNB{i}", [D, ZC_MAX, NX], bf16) for i in range(3)]

            b_eps = mktile("b_eps", [D, 1], f32)
            b_lneta = mktile("b_lneta", [D, 1], f32)
            b_kn = mktile("b_kn", [D, 1], f32)
            zt = mktile("zt", [D, NX], bf16)
            nc.vector.memset(b_eps[:], 1e-8)
            nc.vector.memset(b_lneta[:], math.log(ETA))
            nc.vector.memset(b_kn[:], KN)
            nc.vector.memset(zt[:], 0.0)

            for f in range(3):
                nc.sync.dma_start(scr[f, :, 0, :], zt[0:ZP + 2, :])
                nc.sync.dma_start(scr[f, :, NY - 1, :], zt[0:ZP + 2, :])

            def load(tile_t, src_ap):
                nc.sync.dma_start(tile_t, src_ap.rearrange("z y x -> y z x"))

            def emit_combo(zc, sh, npos, nvelb, mpos, mvelb):
                shz, shy, shx = sh
                v = -shy
                z0, x0 = 1 - shz, 1 - shx
                msl = (slice(None), slice(z0, z0 + zc), slice(x0, x0 + D))
                nsl = (slice(None), slice(1, 1 + zc), slice(1, 1 + D))
                w = slice(0, zc)
                I = {k: t[:, w, :] for k, t in inter.items()}
                fx, fy, fz = FX[:, w, :], FY[:, w, :], FZ[:, w, :]

                tt = nc.vector.tensor_tensor
                ts = nc.vector.tensor_scalar
                gp = nc.gpsimd.tensor_tensor
                act = nc.scalar.activation

                # geometry: fp32
                tt(I["dX"], npos[0][nsl], mpos[0](v)[msl], A.subtract)
                tt(I["dY"], npos[1][nsl], mpos[1](v)[msl], A.subtract)
                tt(I["dZ"], npos[2][nsl], mpos[2](v)[msl], A.subtract)
                act(I["QA"], I["dX"], AF.Square)
                act(I["QB"], I["dY"], AF.Square)
                act(I["QC"], I["dZ"], AF.Square)
                act(I["dXb"], I["dX"], AF.Copy)
                act(I["dYb"], I["dY"], AF.Copy)
                act(I["dZb"], I["dZ"], AF.Copy)
                tt(I["S"], I["QA"], I["QB"], A.add)
                tt(I["S"], I["S"], I["QC"], A.add)
                act(I["LN"], I["S"], AF.Ln, bias=b_eps[:], scale=1.0)
                act(I["R"], I["LN"], AF.Exp, bias=0.0, scale=-0.5)
                act(I["R2"], I["LN"], AF.Exp, bias=b_lneta[:], scale=-1.0)
                # spring term: KN - KN*PS*r  (ACT affine)
                act(I["Aa"], I["R"], AF.Identity, bias=b_kn[:], scale=-KN * PS)
                # contact gate on s directly: [dist < PS] == [s < PS^2]
                ts(I["G"], I["S"], PS * PS, None, A.is_lt)
                # damping: bf16
                tt(I["VA"], nvelb[0][nsl], mvelb[0](v)[msl], A.subtract)
                tt(I["VB"], nvelb[1][nsl], mvelb[1](v)[msl], A.subtract)
                tt(I["VC"], nvelb[2][nsl], mvelb[2](v)[msl], A.subtract)
                tt(I["P1"], I["VA"], I["dXb"], A.mult)
                tt(I["P2"], I["VB"], I["dYb"], A.mult)
                tt(I["TX"], I["VC"], I["dZb"], A.mult)
                tt(I["P1"], I["P1"], I["P2"], A.add)
                tt(I["P1"], I["P1"], I["TX"], A.add)
                tt(I["P2"], I["P1"], I["R2"], A.mult)   # ETA * dvn_raw / s
                tt(I["P3"], I["Aa"], I["P2"], A.add)
                tt(I["C"], I["P3"], I["G"], A.mult)     # gated coef (bf16 out)
                tt(I["TX"], I["C"], I["dXb"], A.mult)
                tt(I["TY"], I["C"], I["dYb"], A.mult)
                tt(I["TZ"], I["C"], I["dZb"], A.mult)
                tt(fx, fx, I["TX"], A.add)
                tt(fy, fy, I["TY"], A.add)
                tt(fz, fz, I["TZ"], A.add)

            def boundary_and_update(zc, nsrc, write_bf16):
                tt = nc.vector.tensor_tensor
                ts = nc.vector.tensor_scalar
                w = slice(0, zc)
                mkw = mk[:, w, 1:1 + D]
                Aa = inter["Aa"][:, w, :]
                G = inter["G"][:, w, :]
                P = inter["P3"][:, w, :]
                T = inter["dX"][:, w, :]
                C = inter["dY"][:, w, :]
                S = inter["S"][:, w, :]
                for comp, (FF, grav) in enumerate(
                        [(FX, 0.0), (FY, 0.0), (FZ, GRAV)]):
                    p = nsrc[comp][:, 1:1 + zc, 1:1 + D]
                    vv = nsrc[3 + comp][:, 1:1 + zc, 1:1 + D]
                    f = FF[:, w, :]
                    ts(Aa, p, PS, None, A.is_gt)
                    ts(G, p, BLO_HI, None, A.is_lt)
                    tt(Aa, Aa, G, A.mult)            # lo
                    ts(G, p, BHI_TH, None, A.is_gt)  # hi
                    ts(T, p, -KN, KN * BLO_HI, A.mult, A.add)
                    tt(T, T, Aa, A.mult)
                    ts(C, p, -KN, KN * BHI_TH, A.mult, A.add)
                    tt(C, C, G, A.mult)
                    tt(T, T, C, A.add)
                    tt(Aa, Aa, G, A.add)             # lo + hi
                    tt(P, vv, Aa, A.mult)
                    ts(P, P, -ETA, None, A.mult)
                    tt(T, T, P, A.add)               # fb
                    tt(S, T, f, A.subtract)
                    if grav != 0.0:
                        ts(S, S, 1.0, grav, A.mult, A.add)
                    tt(S, S, mkw, A.mult)
                    ts(S, S, DT / PM, None, A.mult)
                    vn = VN[comp][:, w, 1:1 + D]
                    tt(vn, vv, S, A.add)
                    if write_bf16:
                        vnb = VNB[comp][:, w, 1:1 + D]
                        tt(vnb, vv, S, A.add)

            def phase(n, chunks, m_list, vel_src):
                """vel_src[m] -> ('ext', layer) or ('scr',) for bf16 vel loads"""
                for (w0, zc) in chunks:
                    zlo, zhi = w0 - 1, w0 + zc + 1
                    for f in range(3):
                        load(nt[f][:, 0:zc + 2, :],
                             ext[POS_NAMES[f]][n, zlo:zhi, 1:1 + D, :])
                        load(nt[3 + f][:, 0:zc + 2, :],
                             ext[VEL_NAMES[f]][n, zlo:zhi, 1:1 + D, :])
                        load(ntb[f][:, 0:zc + 2, :],
                             extb[VEL_NAMES[f]][n, zlo:zhi, 1:1 + D, :])
                    load(mk[:, 0:zc, :], ext["mk"][n, w0:w0 + zc, 1:1 + D, :])
                    nc.gpsimd.memset(FX[:, 0:zc, :], 0.0)
                    nc.gpsimd.memset(FY[:, 0:zc, :], 0.0)
                    nc.gpsimd.memset(FZ[:, 0:zc, :], 0.0)

                    npos = [nt[0], nt[1], nt[2]]
                    nvelb = ntb
                    for m in m_list:
                        same = (m == n)
                        for f in range(3):
                            vs = ((-1, 1) if same else (-1, 0, 1))
                            for v in vs:
                                load(mp[(f, v)][:, 0:zc + 2, :],
                                     ext[POS_NAMES[f]][m, zlo:zhi, 1 + v:1 + v + D, :])
                            src = vel_src[m]
                            for v in (-1, 0, 1):
                                if src[0] == "ext":
                                    load(mv[(f, v)][:, 0:zc + 2, :],
                                         extb[VEL_NAMES[f]][m, zlo:zhi,
                                                            1 + v:1 + v + D, :])
                                else:
                                    load(mv[(f, v)][:, 0:zc + 2, :],
                                         scr[f, w0 - 2:w0 + zc,
                                             1 + v:1 + v + D, :])
                        if same:
                            mpos = [(lambda f_: (lambda v: nt[f_] if v == 0
                                                 else mp[(f_, v)]))(f)
                                    for f in range(3)]
                        else:
                            mpos = [(lambda f_: (lambda v: mp[(f_, v)]))(f)
                                    for f in range(3)]
                        mvelb = [(lambda f_: (lambda v: mv[(f_, v)]))(f)
                                 for f in range(3)]
                        for sh in SHIFTS:
                            if same and sh == (0, 0, 0):
                                continue
                            emit_combo(zc, sh, npos, nvelb, mpos, mvelb)

                    for i in range(3):
                        nc.gpsimd.memset(VN[i][:, 0:zc, :], 0.0)
                        if n == 0:
                            nc.gpsimd.memset(VNB[i][:, 0:zc, :], 0.0)
                    boundary_and_update(zc, nt, write_bf16=(n == 0))
                    if n == 0:
                        for i in range(3):
                            nc.sync.dma_start(
                                scr[i, w0 - 1:w0 - 1 + zc, 1:1 + D, :]
                                .rearrange("z y x -> y z x"),
                                VNB[i][:, 0:zc, :])
                        olo, ohi = max(w0, 2), min(w0 + zc, 2 + ZP)
                        if ohi > olo:
                            for i in range(3):
                                nc.sync.dma_start(
                                    out_ext[i, olo - 2:ohi - 2, :, :]
                                    .rearrange("z y x -> y z x"),
                                    VN[i][:, olo - w0:ohi - w0, 1:1 + D])
                    else:
                        for i in range(3):
                            nc.sync.dma_start(
                                out_ext[3 + i, w0 - 2:w0 - 2 + zc, :, :]
                                .rearrange("z y x -> y z x"),
                                VN[i][:, 0:zc, 1:1 + D])

            # phase 0: n=0; m=0 (same layer) then m=1, vel from ext bf16
            phase(0, [(1, 6), (7, 6), (13, 6)], [0, 1],
                  {0: ("ext",), 1: ("ext",)})
            # phase 1: n=1; m=1 (same layer) then m=0 with vel from scratch
            phase(1, [(2, 6), (8, 6), (14, 4)], [1, 0],
                  {1: ("ext",), 0: ("scr",)})

    nc.compile()
    return nc


def _get_compiled():
    global _compiled
    if _compiled is None:
        _compiled = _build()
    return _compiled


def _pad_field(a, val):
    a = np.ascontiguousarray(a.reshape(2, D, D, D), dtype=np.float32)
    return np.pad(a, ((0, 0), (2, 2), (1, 1), (1, 1)), constant_values=val)


def kernel(x_grid, y_grid, z_grid, vx_grid, vy_grid, vz_grid, mask):
    import ml_dtypes
    from concourse.bass_utils import run_bass_kernel_spmd

    nc = _get_compiled()

    padded = {
        "xg": _pad_field(x_grid, SENT),
        "yg": _pad_field(y_grid, SENT),
        "zg": _pad_field(z_grid, SENT),
        "vx": _pad_field(vx_grid, 0.0),
        "vy": _pad_field(vy_grid, 0.0),
        "vz": _pad_field(vz_grid, 0.0),
        "mk": _pad_field(mask, 0.0),
    }
    for f in VEL_NAMES:
        padded[f + "b"] = padded[f].astype(ml_dtypes.bfloat16)

    in_maps = []
    for c in range(NCORES):
        z0 = ZP * c
        in_maps.append({k: np.ascontiguousarray(v[:, z0:z0 + NZIN])
                        for k, v in padded.items()})

    res = run_bass_kernel_spmd(nc, in_maps, core_ids=list(range(NCORES)))

    out = np.empty((3, 2, 1, 1, D, D, D), np.float32)
    for c in range(NCORES):
        o = res.results[c]["out"]
        z0 = ZP * c
        for comp in range(3):
            out[comp, 0, 0, 0, z0:z0 + ZP] = o[comp]
            out[comp, 1, 0, 0, z0:z0 + ZP] = o[3 + comp]
    return out
